# revision 2
# baseline (speedup 1.0000x reference)
"""Trainium2 Bass kernel for nn_DecoderLayer_33758442946809 (v3).

Sharding: cores 0-3 own batch 0, cores 4-7 batch 1. Self-attention is
HEAD-parallel within each 4-core group: core (g, m) computes heads
4m..4m+3 over the full 2048-token sequence, so K/V projections are done
once (not 4x per token as in pure seq-parallel), and exact causal
chunking skips ~47% of score/exp work with an identical SPMD graph on
every core. Scores of head h+1 are interleaved with the AV matmuls of
head h so the PE never drains while the scalar engine's exp lags (PE
p-state stays at 2.4GHz). Two pipelined 4-core AllGathers (head pairs
01 then 23) redistribute the attention output back to sequence
sharding; a tiny dummy collective at kernel start absorbs the ~90us
one-time CC-pipe init, and W1 starts accumulating the AG0 half while
AG1 flies. The per-core 512 rows are selected from the gathered buffer
with a partition_id-derived register-offset DMA. Phases 3-6 (W1+LN1,
cross-attention, W2sum+LN2, FFN+LN3) are sequence-parallel over 512-row
shards.

Tricks: tile(attn2, H) @ W2 == attn2 @ sum_h W2[h] (host-precomputed
block sum); softmax without max-subtraction (logits provably bounded),
denominators via an appended ones-column in V (self) / a ones-vector
matmul (cross); SCALE folded into ln1_g/ln1_b and the pre-LN biases
b1/b2/bf2 folded into the residual y rows host-side; LN outputs written
bf16 so PE transposes run at 1 cycle/row; softmax/LN reciprocals use
the fast custom-DVE approximation (18 bits).
"""
import math
import sys

import numpy as np

sys.path.insert(0, "/opt/trn_rl_repo")

import ml_dtypes  # noqa: E402

import concourse.bass as bass  # noqa: E402
import concourse.tile as tile  # noqa: E402
from concourse import bacc, mybir  # noqa: E402
from concourse.bass_utils import run_bass_kernel_spmd  # noqa: E402
from concourse.masks import make_identity  # noqa: E402

B, S, D, H, DF = 2, 2048, 1024, 16, 4096
DK = D // H                      # 64
P = 128
R = 512                          # rows per core in seq-parallel phases
T = S
KC = D // P                      # 8
TB = T // P                      # 16 key blocks
RB = R // P                      # 4
FB = DF // P                     # 32
HL = 4                           # heads per core in attention phases
NCORES = 8
SCALE = 1.0 / math.sqrt(DK)
GROUPS = [[0, 1, 2, 3], [4, 5, 6, 7]]

F32 = mybir.dt.float32
BF16 = mybir.dt.bfloat16
AF = mybir.ActivationFunctionType
ALU = mybir.AluOpType

_cached = {}
DEBUG = False


def _ln_rows(nc, pool, x_ap, out_ap, eps_sb, g_b, be_b):
    """LayerNorm along the free axis (D) of a token-major [128, D] f32
    tile; the final affine writes `out_ap` (any dtype)."""
    x3 = x_ap.rearrange("p (n f) -> p n f", f=512)
    stats = pool.tile([P, 2, 6], F32, name="ln_stats", tag="ln_stats", bufs=4)
    for sg in range(2):
        nc.vector.bn_stats(out=stats[:, sg, :], in_=x3[:, sg, :])
    mv = pool.tile([P, 2], F32, name="ln_mv", tag="ln_mv", bufs=4)
    nc.vector.bn_aggr(out=mv[:], in_=stats[:])
    std = pool.tile([P, 1], F32, name="ln_std", tag="ln_std", bufs=4)
    nc.scalar.activation(out=std[:], in_=mv[:, 1:2], func=AF.Sqrt,
                         bias=eps_sb[:], scale=1.0)
    rstd = pool.tile([P, 1], F32, name="ln_rstd", tag="ln_rstd", bufs=4)
    nc.vector.reciprocal(out=rstd[:], in_=std[:])
    nc.vector.tensor_scalar(out=x_ap, in0=x_ap, scalar1=mv[:, 0:1],
                            scalar2=rstd[:], op0=ALU.subtract, op1=ALU.mult)
    nc.vector.tensor_mul(out=x_ap, in0=x_ap, in1=g_b)
    nc.vector.tensor_add(out=out_ap, in0=x_ap, in1=be_b)


def build_nc():
    nc = bacc.Bacc("TRN2", target_bir_lowering=False, debug=False,
                   num_devices=NCORES)

    dram = {}

    def din(name, shape, dt):
        dram[name] = nc.dram_tensor(name, shape, dt, kind="ExternalInput").ap()

    din("yT", [D, T], BF16)          # y[g].T
    din("xT", [D, T], BF16)          # x[g].T
    din("x_tm", [T, D], BF16)        # x[g]
    din("mask_tri", [P, P], BF16)    # triu causal mask for diagonal blocks
    din("Wq_fm", [D, HL * DK], BF16)
    din("Wk_fm", [D, HL * DK], BF16)
    din("Wv_fm", [D, HL * DK], BF16)
    din("bq_s", [HL * DK], F32)      # bq * SCALE for local heads
    din("bk_f", [HL * DK], F32)
    din("bv_f", [HL * DK], F32)
    din("W1", [D, D], BF16)
    din("y1_rows", [R, D], F32)      # y rows + b1
    din("ln1_g", [D], F32)           # * SCALE
    din("ln1_b", [D], F32)           # * SCALE
    din("W2sum", [D, D], BF16)
    din("y2_rows", [R, D], F32)      # y rows + b2
    din("ln2_g", [D], F32)
    din("ln2_b", [D], F32)
    din("Wf1", [D, DF], BF16)
    din("bf1", [DF], F32)
    din("Wf2", [DF, D], BF16)
    din("y3_rows", [R, D], F32)      # y rows + bf2
    din("ln3_g", [D], F32)
    din("ln3_b", [D], F32)
    out_d = nc.dram_tensor("out", [R, D], F32, kind="ExternalOutput").ap()
    dbg = {}
    if DEBUG:
        dbg["catT"] = nc.dram_tensor("dbg_catT", [2 * P, T], BF16,
                                     kind="ExternalOutput").ap()
        dbg["catF"] = nc.dram_tensor("dbg_catF", [KC * P, R], BF16,
                                     kind="ExternalOutput").ap()

    with tile.TileContext(nc) as tc:
        _build(nc, tc, dram, out_d, dbg)
    nc.compile()
    return nc


def _build(nc, tc, d, out_d, dbg):
    pool_cms = {}

    def open_pool(*args, **kw):
        cm = tc.tile_pool(*args, **kw)
        p = cm.__enter__()
        pool_cms[id(p)] = cm
        return p

    def close_pool(p):
        pool_cms.pop(id(p)).__exit__(None, None, None)

    const = open_pool(name="const", bufs=1, side="left")
    dramp = open_pool(name="dramp", bufs=1, space="DRAM")

    # -- tiny warmup collective: absorbs the one-time CC-pipe init cost
    wu_i = dramp.tile([1, P], BF16, name="wu_i", tag="wu_i")
    wu_o = dramp.tile([4, P], BF16, name="wu_o", tag="wu_o")
    zz = const.tile([1, P], BF16, name="zz", tag="zz")
    nc.vector.memset(zz[:], 0.0)
    nc.gpsimd.dma_start(out=wu_i[:], in_=zz[:])
    nc.gpsimd.collective_compute(
        "AllGather", ALU.bypass, replica_groups=GROUPS,
        ins=[wu_i.opt()], outs=[wu_o.opt()])

    ident_bf = const.tile([P, P], BF16, name="ident_bf", tag="ident_bf")
    make_identity(nc, ident_bf[:])
    ones_bf = const.tile([P, 1], BF16, name="ones_bf", tag="ones_bf")
    nc.vector.memset(ones_bf[:], 1.0)
    eps_sb = const.tile([P, 1], F32, name="eps", tag="eps")
    nc.vector.memset(eps_sb[:], 1e-5)
    mtri = const.tile([P, P], BF16, name="mtri", tag="mtri")
    nc.sync.dma_start(out=mtri[:], in_=d["mask_tri"])

    def bias_chunks(pool, name, n):
        t = pool.tile([P, n], F32, name=f"bc_{name}", tag=f"bc_{name}")
        nc.sync.dma_start(out=t[:], in_=d[name].rearrange("(n p) -> p n", p=P))
        return t

    def bcast_row(pool, name, w=D):
        src = d[name]
        t = pool.tile([P, w], F32, name=f"br_{name}", tag=f"br_{name}")
        bc = bass.AP(tensor=src.tensor, offset=src.offset,
                     ap=[[0, P]] + list(src.ap))
        nc.sync.dma_start(out=t[:], in_=bc)
        return t

    bq_sb = bias_chunks(const, "bq_s", 2)
    bk_sb = bias_chunks(const, "bk_f", 2)
    bf1_sb = bias_chunks(const, "bf1", FB)

    cat_in = [dramp.tile([P, T], BF16, name=f"cat_in{i}", tag=f"cat_in{i}")
              for i in range(2)]
    cat_out = [dramp.tile([4 * P, T], BF16, name=f"cat_out{i}",
                          tag=f"cat_out{i}") for i in range(2)]

    # ================= Phase 1: Q/K/V projections (4 heads) ============
    attn = open_pool(name="attn", bufs=1, side="left")
    qT = [attn.tile([P, T], BF16, name=f"qT{i}", tag=f"qT{i}") for i in range(2)]
    kT = [attn.tile([P, T], BF16, name=f"kT{i}", tag=f"kT{i}") for i in range(2)]
    v_sb = [attn.tile([P, HL, DK + 1], BF16, name=f"v{i}", tag=f"v{i}")
            for i in range(TB)]
    catT = [attn.tile([P, T], BF16, name=f"catT{i}", tag=f"catT{i}")
            for i in range(2)]
    ypool = open_pool(name="ypool", bufs=1, side="left")
    yTt = [ypool.tile([P, 2, T], BF16, name=f"yT{i}", tag=f"yT{i}")
           for i in range(4)]
    qkw = open_pool(name="qkw", bufs=1, side="left")
    wq = qkw.tile([P, KC, HL * DK], BF16, name="wq", tag="wq")
    wk = qkw.tile([P, KC, HL * DK], BF16, name="wk", tag="wk")
    wv = qkw.tile([P, KC, HL * DK], BF16, name="wv", tag="wv")
    for i in range(4):
        nc.sync.dma_start(out=yTt[i][:], in_=d["yT"][i * 256:(i + 1) * 256, :]
                          .rearrange("(a p) c -> p a c", p=P))
    nc.sync.dma_start(out=wq[:], in_=d["Wq_fm"].rearrange("(c p) n -> p c n", p=P))
    nc.sync.dma_start(out=wk[:], in_=d["Wk_fm"].rearrange("(c p) n -> p c n", p=P))
    nc.sync.dma_start(out=wv[:], in_=d["Wv_fm"].rearrange("(c p) n -> p c n", p=P))
    bv_b = bcast_row(qkw, "bv_f", HL * DK)

    def yT(kc):
        return yTt[kc // 2][:, kc % 2, :]

    psQ = open_pool(name="psQ", bufs=3, space="PSUM", side="left")
    for ft in range(2):
        for rs in range(4):
            ps = psQ.tile([P, 512], F32, name="psq", tag="psq")
            for kc in range(KC):
                nc.tensor.matmul(ps[:], lhsT=wq[:, kc, ft * P:(ft + 1) * P],
                                 rhs=yT(kc)[:, rs * 512:(rs + 1) * 512],
                                 start=(kc == 0), stop=(kc == KC - 1))
            nc.scalar.activation(out=qT[ft][:, rs * 512:(rs + 1) * 512],
                                 in_=ps[:], func=AF.Identity,
                                 bias=bq_sb[:, ft:ft + 1], scale=SCALE)
        for rs in range(4):
            ps = psQ.tile([P, 512], F32, name="psq", tag="psq")
            for kc in range(KC):
                nc.tensor.matmul(ps[:], lhsT=wk[:, kc, ft * P:(ft + 1) * P],
                                 rhs=yT(kc)[:, rs * 512:(rs + 1) * 512],
                                 start=(kc == 0), stop=(kc == KC - 1))
            nc.scalar.activation(out=kT[ft][:, rs * 512:(rs + 1) * 512],
                                 in_=ps[:], func=AF.Identity,
                                 bias=bk_sb[:, ft:ft + 1], scale=1.0)
    psV = open_pool(name="psV", bufs=2, space="PSUM", side="left")
    for kb in range(TB):
        nc.vector.memset(v_sb[kb][:, :, DK:DK + 1], 1.0)
        ps = psV.tile([P, HL * DK], F32, name="psv", tag="psv")
        for kc in range(KC):
            nc.tensor.matmul(ps[:], lhsT=yT(kc)[:, kb * P:(kb + 1) * P],
                             rhs=wv[:, kc, :],
                             start=(kc == 0), stop=(kc == KC - 1))
        nc.vector.tensor_add(
            out=v_sb[kb][:, :, 0:DK],
            in0=ps[:].rearrange("p (h k) -> p h k", h=HL),
            in1=bv_b[:, :].rearrange("p (h k) -> p h k", h=HL))
    close_pool(psV)
    close_pool(psQ)
    close_pool(qkw)
    close_pool(ypool)

    # Early phase-3 buffers + weight prefetch (right side; the W1/y1
    # loads overlap phase 2 so W1 can start the moment AG0 lands).
    a1p = open_pool(name="a1p", bufs=1, side="right")
    a1T = [a1p.tile([P, R], BF16, name=f"a1T{i}", tag=f"a1T{i}") for i in range(KC)]
    catp = open_pool(name="catp", bufs=1, side="right")
    catF = [catp.tile([P, R], BF16, name=f"catF{i}", tag=f"catF{i}")
            for i in range(KC)]
    ph3 = open_pool(name="ph3", bufs=1, side="right")
    w1t = ph3.tile([P, KC, D], BF16, name="w1t", tag="w1t")
    nc.sync.dma_start(out=w1t[:], in_=d["W1"].rearrange("(c p) n -> p c n", p=P))
    y1t = ph3.tile([P, RB, D], F32, name="y1t", tag="y1t")
    nc.sync.dma_start(out=y1t[:], in_=d["y1_rows"].rearrange("(a p) c -> p a c", p=P))
    g1_b = bcast_row(ph3, "ln1_g")
    be1_b = bcast_row(ph3, "ln1_b")

    def read_catF(par):
        # par=0: even kc blocks from AG0; par=1: odd from AG1
        m4 = nc.sync.partition_id() % 4
        ob_ap = cat_out[par][:]
        for s in range(4):
            src = bass.AP(
                tensor=ob_ap.tensor,
                offset=ob_ap.offset + s * P * T + m4 * R,
                ap=[[T, P], [1, R]])
            nc.sync.dma_start(out=catF[2 * s + par][:], in_=src)

    # ====== Phase 2: causal self-attention, head-parallel over seq =====
    # scores(h) emission is interleaved with AV(h-1) so the PE always has
    # runnable matmuls while exp drains on the scalar engine.
    ph2 = open_pool(name="ph2", bufs=1, side="left")
    psS = open_pool(name="psS", bufs=3, space="PSUM", side="left")
    psAV = open_pool(name="psAV", bufs=2, space="PSUM", side="left")

    exp_tiles = {}

    def scores_kb(h, kb):
        ht, ho = h // 2, (h % 2) * DK
        rs0 = kb // 4
        fill0 = (kb % 4) * P
        ncols_t = (4 - rs0) * 512            # strip-aligned tile width
        et = ph2.tile([P, ncols_t], BF16, name=f"expT{kb}", tag=f"expT{kb}",
                      bufs=2)
        exp_tiles[(h, kb)] = et
        if fill0:
            nc.gpsimd.memset(et[:, 0:fill0], 0.0)
        ncols = ncols_t - fill0              # exact causal cols
        for g in range((ncols + 1023) // 1024):
            gcols = min(1024, ncols - g * 1024)
            ps = psS.tile([P, 1024], F32, name="ps_s", tag="ps_s")
            for seg in range((gcols + 511) // 512):
                scols = min(512, gcols - seg * 512)
                c0 = kb * P + g * 1024 + seg * 512
                nc.tensor.matmul(
                    ps[:, seg * 512:seg * 512 + scols],
                    lhsT=kT[ht][ho:ho + DK, kb * P:(kb + 1) * P],
                    rhs=qT[ht][ho:ho + DK, c0:c0 + scols],
                    start=True, stop=True)
            nc.scalar.activation(
                out=et[:, fill0 + g * 1024:fill0 + g * 1024 + gcols],
                in_=ps[:, 0:gcols], func=AF.Exp)
        nc.vector.tensor_mul(out=et[:, fill0:fill0 + P],
                             in0=et[:, fill0:fill0 + P], in1=mtri[:])

    def av_rs(h, rs):
        ht, ho = h // 2, (h % 2) * DK
        pa = psAV.tile([DK + 1, 512], F32, name="pa", tag="pa")
        for kb in range(4 * rs + 4):
            off = (rs - kb // 4) * 512
            nc.tensor.matmul(pa[:], lhsT=v_sb[kb][:, h, :],
                             rhs=exp_tiles[(h, kb)][:, off:off + 512],
                             start=(kb == 0), stop=(kb == 4 * rs + 3))
        recip = ph2.tile([1, 512], F32, name="recip", tag="recip", bufs=2)
        nc.vector.reciprocal(out=recip[:], in_=pa[DK:DK + 1, :])
        recipb = ph2.tile([DK, 512], F32, name="recipb", tag="recipb", bufs=2)
        nc.gpsimd.partition_broadcast(recipb[:], recip[:])
        nc.vector.tensor_mul(
            out=catT[ht][ho:ho + DK, rs * 512:(rs + 1) * 512],
            in0=pa[0:DK, :], in1=recipb[:])

    def launch_ag(i):
        nc.gpsimd.dma_start(out=cat_in[i][:], in_=catT[i][:])
        nc.gpsimd.collective_compute(
            "AllGather", ALU.bypass, replica_groups=GROUPS,
            ins=[cat_in[i].opt()], outs=[cat_out[i].opt()])

    for kb in range(TB):
        scores_kb(0, kb)
    for h in (1, 2, 3):
        for rs in range(4):
            for kb in range(4 * rs, 4 * rs + 4):
                scores_kb(h, kb)
            av_rs(h - 1, rs)
        if h == 2:
            launch_ag(0)
            read_catF(0)
    for rs in range(4):
        av_rs(3, rs)
    launch_ag(1)
    read_catF(1)
    if DEBUG:
        for i in range(2):
            nc.sync.dma_start(out=dbg["catT"][i * P:(i + 1) * P, :],
                              in_=catT[i][:])
        for kc in range(KC):
            nc.sync.dma_start(out=dbg["catF"][kc * P:(kc + 1) * P, :],
                              in_=catF[kc][:])

    close_pool(psAV)
    close_pool(psS)
    close_pool(ph2)
    close_pool(attn)

    # prefetch cross-attention inputs on the gpsimd queue
    xpool = open_pool(name="xpool", bufs=1, side="left")
    xTt = [xpool.tile([P, 2, T], BF16, name=f"xT{i}", tag=f"xT{i}")
           for i in range(4)]
    for i in range(4):
        nc.gpsimd.dma_start(out=xTt[i][:], in_=d["xT"][i * 256:(i + 1) * 256, :]
                            .rearrange("(a p) c -> p a c", p=P))
    xmt = [xpool.tile([P, 4, D], BF16, name=f"xtm{i}", tag=f"xtm{i}")
           for i in range(4)]
    for i in range(4):
        nc.gpsimd.dma_start(out=xmt[i][:], in_=d["x_tm"][i * 512:(i + 1) * 512, :]
                            .rearrange("(a p) c -> p a c", p=P))

    def xT(kc):
        return xTt[kc // 2][:, kc % 2, :]

    def x_tm(tb):
        return xmt[tb // 4][:, tb % 4, :]

    # ========= Phase 3: W1 + residual + LN1, produce a1T (prescaled) ===
    pp3 = open_pool(name="pp3", bufs=6, space="PSUM", side="right")
    pt3 = open_pool(name="pt3", bufs=2, space="PSUM", side="right")
    kc_order = [0, 2, 4, 6, 1, 3, 5, 7]
    for rb in range(RB):
        a1 = ph3.tile([P, D], F32, name="a1", tag="a1", bufs=2)
        a1_bf = ph3.tile([P, D], BF16, name="a1b", tag="a1b", bufs=2)
        for nt in range(2):
            ps = pp3.tile([P, 512], F32, name="ps_a1", tag="ps_a1")
            for i, kc in enumerate(kc_order):
                nc.tensor.matmul(ps[:],
                                 lhsT=catF[kc][:, rb * P:(rb + 1) * P],
                                 rhs=w1t[:, kc, nt * 512:(nt + 1) * 512],
                                 start=(i == 0), stop=(i == KC - 1))
            sl = slice(nt * 512, (nt + 1) * 512)
            nc.vector.tensor_add(out=a1[:, sl], in0=ps[:], in1=y1t[:, rb, sl])
        _ln_rows(nc, ph3, a1[:], a1_bf[:], eps_sb, g1_b[:], be1_b[:])
        for kc in range(KC):
            pt = pt3.tile([P, P], BF16, name="pt_a1", tag="pt_a1")
            nc.tensor.transpose(pt[:], a1_bf[:, kc * P:(kc + 1) * P], ident_bf[:])
            nc.scalar.copy(out=a1T[kc][:, rb * P:(rb + 1) * P], in_=pt[:])
    close_pool(pt3)
    close_pool(pp3)
    close_pool(ph3)
    close_pool(catp)

    # ================= Phase 4: cross-attention =======================
    at2p = open_pool(name="at2p", bufs=1, side="left")
    at2T = [at2p.tile([P, R], BF16, name=f"at2T{i}", tag=f"at2T{i}") for i in range(KC)]

    ph4 = open_pool(name="ph4", bufs=1, side="left")
    pp4 = open_pool(name="pp4", bufs=4, space="PSUM", side="left")
    pd4 = open_pool(name="pd4", bufs=1, space="PSUM", side="left")
    p2T = [ph4.tile([P, R], BF16, name=f"p2T{i}", tag=f"p2T{i}") for i in range(TB)]
    for tb in range(TB):
        ps = pp4.tile([P, 512], F32, name="ps4", tag="ps4")
        for kc in range(KC):
            nc.tensor.matmul(ps[:], lhsT=xT(kc)[:, tb * P:(tb + 1) * P],
                             rhs=a1T[kc][:, :],
                             start=(kc == 0), stop=(kc == KC - 1))
        nc.scalar.activation(out=p2T[tb][:], in_=ps[:], func=AF.Exp)
    pd = pd4.tile([1, R], F32, name="ps_d2", tag="ps_d2")
    for tb in range(TB):
        nc.tensor.matmul(pd[:], lhsT=ones_bf[:], rhs=p2T[tb][:],
                         start=(tb == 0), stop=(tb == TB - 1))
    recip2 = ph4.tile([1, R], F32, name="recip2", tag="recip2")
    nc.vector.reciprocal(out=recip2[:], in_=pd[:])
    recip2b = ph4.tile([P, R], F32, name="recip2b", tag="recip2b")
    nc.gpsimd.partition_broadcast(recip2b[:], recip2[:])
    for db in range(KC):
        ps = pp4.tile([P, 512], F32, name="ps4", tag="ps4")
        for tb in range(TB):
            nc.tensor.matmul(ps[:], lhsT=x_tm(tb)[:, db * P:(db + 1) * P],
                             rhs=p2T[tb][:],
                             start=(tb == 0), stop=(tb == TB - 1))
        nc.vector.tensor_mul(out=at2T[db][:], in0=ps[:], in1=recip2b[:])
    close_pool(pd4)
    close_pool(pp4)
    close_pool(ph4)
    close_pool(a1p)

    # ========= Phase 5: W2sum + residual + LN2, produce a2T ===========
    a2p = open_pool(name="a2p", bufs=1, side="right")
    a2T = [a2p.tile([P, R], BF16, name=f"a2T{i}", tag=f"a2T{i}") for i in range(KC)]

    ph5 = open_pool(name="ph5", bufs=1, side="right")
    pp5 = open_pool(name="pp5", bufs=4, space="PSUM", side="right")
    pt5 = open_pool(name="pt5", bufs=2, space="PSUM", side="right")
    w2t = ph5.tile([P, KC, D], BF16, name="w2t", tag="w2t")
    nc.sync.dma_start(out=w2t[:], in_=d["W2sum"].rearrange("(c p) n -> p c n", p=P))
    y2t = ph5.tile([P, RB, D], F32, name="y2t", tag="y2t")
    nc.sync.dma_start(out=y2t[:], in_=d["y2_rows"].rearrange("(a p) c -> p a c", p=P))
    g2_b = bcast_row(ph5, "ln2_g")
    be2_b = bcast_row(ph5, "ln2_b")
    for rb in range(RB):
        a2 = ph5.tile([P, D], F32, name="a2", tag="a2", bufs=2)
        a2_bf = ph5.tile([P, D], BF16, name="a2b", tag="a2b", bufs=2)
        for nt in range(2):
            ps = pp5.tile([P, 512], F32, name="ps_a2", tag="ps_a2")
            for kc in range(KC):
                nc.tensor.matmul(ps[:],
                                 lhsT=at2T[kc][:, rb * P:(rb + 1) * P],
                                 rhs=w2t[:, kc, nt * 512:(nt + 1) * 512],
                                 start=(kc == 0), stop=(kc == KC - 1))
            sl = slice(nt * 512, (nt + 1) * 512)
            nc.vector.tensor_add(out=a2[:, sl], in0=ps[:], in1=y2t[:, rb, sl])
        _ln_rows(nc, ph5, a2[:], a2_bf[:], eps_sb, g2_b[:], be2_b[:])
        for kc in range(KC):
            pt = pt5.tile([P, P], BF16, name="pt_a2", tag="pt_a2")
            nc.tensor.transpose(pt[:], a2_bf[:, kc * P:(kc + 1) * P], ident_bf[:])
            nc.scalar.copy(out=a2T[kc][:, rb * P:(rb + 1) * P], in_=pt[:])
    close_pool(pt5)
    close_pool(pp5)
    close_pool(ph5)
    close_pool(at2p)

    # ========== Phase 6: FFN (streamed weights) + residual + LN3 =======
    fA = open_pool(name="fA", bufs=1, side="left")
    f1T = [fA.tile([P, R], BF16, name=f"f1T{i}", tag=f"f1T{i}") for i in range(FB)]
    pfA = open_pool(name="pfA", bufs=3, space="PSUM", side="left")
    wf1_src = d["Wf1"]
    for fb in range(FB):
        wf1_fb = fA.tile([P, KC, P], BF16, name="wf1s", tag="wf1s", bufs=3)
        nc.sync.dma_start(
            out=wf1_fb[:],
            in_=wf1_src[:, fb * P:(fb + 1) * P].rearrange(
                "(c p) n -> p c n", p=P))
        ps = pfA.tile([P, 512], F32, name="ps_f1", tag="ps_f1")
        for kc in range(KC):
            nc.tensor.matmul(ps[:], lhsT=wf1_fb[:, kc, :],
                             rhs=a2T[kc][:, :],
                             start=(kc == 0), stop=(kc == KC - 1))
        nc.scalar.activation(out=f1T[fb][:], in_=ps[:], func=AF.Relu,
                             bias=bf1_sb[:, fb:fb + 1], scale=1.0)
    close_pool(pfA)
    close_pool(a2p)

    pfB = open_pool(name="pfB", bufs=1, space="PSUM", side="left")
    fB = open_pool(name="fB", bufs=1, side="left")
    ps_rb = [pfB.tile([P, D], F32, name=f"ps_rb{i}", tag=f"ps_rb{i}")
             for i in range(RB)]
    for fb in range(FB):
        wf2_fb = fB.tile([P, D], BF16, name="wf2s", tag="wf2s", bufs=3)
        nc.sync.dma_start(out=wf2_fb[:], in_=d["Wf2"][fb * P:(fb + 1) * P, :])
        for rb in range(RB):
            for nt in range(2):
                nc.tensor.matmul(ps_rb[rb][:, nt * 512:(nt + 1) * 512],
                                 lhsT=f1T[fb][:, rb * P:(rb + 1) * P],
                                 rhs=wf2_fb[:, nt * 512:(nt + 1) * 512],
                                 start=(fb == 0), stop=(fb == FB - 1))
    y3t = fB.tile([P, RB, D], F32, name="y3t", tag="y3t")
    nc.sync.dma_start(out=y3t[:], in_=d["y3_rows"].rearrange("(a p) c -> p a c", p=P))
    g3_b = bcast_row(fB, "ln3_g")
    be3_b = bcast_row(fB, "ln3_b")
    for rb in range(RB):
        ff = fB.tile([P, D], F32, name="ff", tag="ff", bufs=2)
        ffo = fB.tile([P, D], F32, name="ffo", tag="ffo", bufs=2)
        nc.vector.tensor_add(out=ff[:], in0=ps_rb[rb][:], in1=y3t[:, rb, :])
        _ln_rows(nc, fB, ff[:], ffo[:], eps_sb, g3_b[:], be3_b[:])
        nc.sync.dma_start(out=out_d[rb * P:(rb + 1) * P, :], in_=ffo[:])
    close_pool(fB)
    close_pool(pfB)
    close_pool(fA)
    close_pool(xpool)
    close_pool(dramp)
    close_pool(const)


def _prep_host(inputs):
    f32 = lambda a: np.ascontiguousarray(np.asarray(a, np.float32))
    bf = lambda a: np.ascontiguousarray(
        np.asarray(a, np.float32).astype(ml_dtypes.bfloat16))
    x = f32(inputs["x"])
    y = f32(inputs["y"])
    pp = np.arange(P)[:, None]
    cc = np.arange(P)[None, :]
    mask_tri = bf((pp <= cc).astype(np.float32))

    Wq = f32(inputs["Wq"])
    Wk = f32(inputs["Wk"])
    Wv = f32(inputs["Wv"])
    b1 = f32(inputs["b1"])
    b2 = f32(inputs["b2"])
    bf2 = f32(inputs["bf2"])
    shared = {
        "mask_tri": mask_tri,
        "W1": bf(inputs["W1"]),
        "ln1_g": f32(inputs["ln1_g"]) * np.float32(SCALE),
        "ln1_b": f32(inputs["ln1_b"]) * np.float32(SCALE),
        "W2sum": bf(f32(inputs["W2"]).reshape(H, D, D).sum(0)),
        "ln2_g": f32(inputs["ln2_g"]), "ln2_b": f32(inputs["ln2_b"]),
        "Wf1": bf(inputs["Wf1"]),
        "bf1": f32(inputs["bf1"]),
        "Wf2": bf(inputs["Wf2"]),
        "ln3_g": f32(inputs["ln3_g"]), "ln3_b": f32(inputs["ln3_b"]),
    }
    in_maps = []
    for c in range(NCORES):
        g, m = c // 4, c % 4
        hs = slice(HL * m, HL * m + HL)
        r0 = m * R
        yr = y[g][r0:r0 + R]
        in_maps.append({
            "yT": bf(y[g].T),
            "xT": bf(x[g].T),
            "x_tm": bf(x[g]),
            "Wq_fm": bf(Wq[hs].transpose(1, 0, 2).reshape(D, HL * DK)),
            "Wk_fm": bf(Wk[hs].transpose(1, 0, 2).reshape(D, HL * DK)),
            "Wv_fm": bf(Wv[hs].transpose(1, 0, 2).reshape(D, HL * DK)),
            "bq_s": f32(inputs["bq"])[hs].reshape(-1) * np.float32(SCALE),
            "bk_f": f32(inputs["bk"])[hs].reshape(-1),
            "bv_f": f32(inputs["bv"])[hs].reshape(-1),
            "y1_rows": np.ascontiguousarray(yr + b1),
            "y2_rows": np.ascontiguousarray(yr + b2),
            "y3_rows": np.ascontiguousarray(yr + bf2),
            **shared,
        })
    return in_maps


def kernel(**inputs):
    if "nc" not in _cached:
        _cached["nc"] = build_nc()
    nc = _cached["nc"]
    in_maps = _prep_host(inputs)
    res = run_bass_kernel_spmd(nc, in_maps, core_ids=list(range(NCORES)))
    out = np.zeros((B, S, D), np.float32)
    for c in range(NCORES):
        g, m = c // 4, c % 4
        out[g, m * R:(m + 1) * R] = res.results[c]["out"]
    return out
